# revision 1
# baseline (speedup 1.0000x reference)
"""Bass/Trainium2 kernel for the 2-layer GAT (nn_GAT_11106785427688).

Strategy (8 NeuronCores, SPMD single NEFF):
- dst-ownership sharding: core c owns nodes [c*OWN, (c+1)*OWN); it receives
  every edge whose dst it owns (~137K edges), so segment-softmax denominators
  and message sums complete locally -- no all-reduce. One AllGather of the
  layer-1 activations between layers; host assembles the final output from
  per-core slices.
- Per-edge gather of packed [h | a_src.h] rows (fp16, 256B) from an HBM table
  via the SWDGE dma_gather custom op (int16 indices -> src buckets of 32768
  rows; table rows permuted so the dense phase writes 2KB-contiguous runs).
- No indexed scatter (HW dma_scatter_add loses duplicate updates): edges are
  grouped by 128-node dst window; one-hot R [edges x nodes] (fp16) built on
  DVE via iota-compare turns segment-sum into PE matmul accumulated in PSUM.
  Softmax division is deferred: out = (sum_e w*h[src]) / (sum_e w).
- exp(leakyrelu(e)) computed without max-subtraction (shift-invariant).
"""
import numpy as np
import ml_dtypes

from concourse import bacc, mybir
import concourse.tile as tile
from concourse.bass_utils import run_bass_kernel_spmd

# ---- problem constants ----
N = 100000
D = 64
H1, C1 = 4, 16
NEG = 0.2
NCORES = 8
OWN = 12544                 # 98 windows * 128 per core
BUCK = 32768
CHUNK = 1024                # gather idxs per dma_gather call (ring limit)
TPC = CHUNK // 128          # tiles per chunk = 8

F16 = mybir.dt.float16
F32 = mybir.dt.float32
BF16 = mybir.dt.bfloat16
I16 = mybir.dt.int16
NPF16 = np.float16
NPBF16 = ml_dtypes.bfloat16


def _derived():
    NW = OWN // 128
    NPAD = NCORES * OWN
    NBUCK = (NPAD + BUCK - 1) // BUCK
    TBL_ROWS = NBUCK * BUCK
    return NW, NPAD, NBUCK, TBL_ROWS


def _perm_row(src):
    """Permuted table row for node src: tb*1024 + p*8 + j (write-friendly)."""
    tb, r = np.divmod(src, 1024)
    j, p = np.divmod(r, 128)
    return tb * 1024 + p * 8 + j


def prep(edge_index):
    """Vectorized host prep: quantile-banded schedule.

    Per-(core,window) edges sorted by src, quantile-spread into the padded
    window group (G_w = roundup128(max-over-cores)). Window tiles are split
    into bands of <=3 tiles; the schedule is band-major so consecutive tiles
    cover the same src-quantile region. Each 1024-slot chunk then spans <=~31
    perm-blocks and gets ONE dma_gather call with a dynamic host-computed
    base (int16 idx). Bands are chunk-aligned (pad tiles trail per band).
    """
    NW, NPAD, NBUCK, TBL_ROWS = _derived()
    # self-loops are handled densely in adw_fill, not in the gather sweep
    src = np.asarray(edge_index[0])
    dst = np.asarray(edge_index[1])
    owner = dst // OWN

    per_core = []
    counts = np.zeros((NCORES, NW), np.int64)
    for c in range(NCORES):
        m = owner == c
        s = src[m]
        d = dst[m] - c * OWN
        w = d >> 7
        order = np.lexsort((s, w))
        s, d, w = s[order], d[order], w[order]
        per_core.append((s, d, w))
        counts[c] = np.bincount(w, minlength=NW)

    gsize = ((counts.max(0) + 127) // 128 * 128).astype(np.int64)   # [NW]
    kw = gsize // 128                                               # tiles/window

    # band-major tile schedule: band b = quantile quarter [b/4,(b+1)/4) of
    # every window, so run centers align across windows regardless of K_w
    NBANDS = 4
    kb = [[int(round(b * int(kw[w]) / NBANDS)) for b in range(NBANDS + 1)]
          for w in range(NW)]
    tile_list = []          # (w, k) in schedule order
    for b in range(NBANDS):
        for w in range(NW):
            for k in range(kb[w][b], kb[w][b + 1]):
                tile_list.append((w, k))
        # chunk-align each band (pad tiles trail inside the band's last chunk)
        while len(tile_list) % TPC != 0:
            tile_list.append((-1, -1))

    n_tiles = len(tile_list)
    total_slots = n_tiles * 128
    n_chunks = total_slots // CHUNK
    tile_w = np.array([w for w, _ in tile_list], np.int64)
    # slot base of each (w,k) tile
    tile_base = {}
    for t, (w, k) in enumerate(tile_list):
        if w >= 0:
            tile_base[(w, k)] = t * 128
    # first/last per (window, band) run
    tile_first = np.zeros(n_tiles, bool)
    tile_last = np.zeros(n_tiles, bool)
    tile_final = np.zeros(n_tiles, bool)
    for t, (w, k) in enumerate(tile_list):
        if w < 0:
            continue
        tile_first[t] = k in [kb[w][b] for b in range(NBANDS)]
        tile_last[t] = (k + 1) in [kb[w][b + 1] for b in range(NBANDS)]
        tile_final[t] = k + 1 == int(kw[w])

    # per-core slot arrays + per-tile block ranges
    idx_h = np.zeros((NCORES, 128, n_chunks * (CHUNK // 16)), np.int16)
    dcol_h = np.zeros((NCORES, 128, n_chunks * TPC), NPF16)
    drow_h = np.zeros((NCORES, 1, n_chunks * CHUNK), NPBF16)
    pr_all = np.zeros((NCORES, total_slots), np.int64)
    off_all = np.full((NCORES, total_slots), -1, np.int64)
    tb_arr = np.full(NW * 32, -1, np.int64)
    for (w, k), sb in tile_base.items():
        tb_arr[w * 32 + k] = sb
    for c in range(NCORES):
        s, d, w = per_core[c]
        grp_first = np.searchsorted(w, np.arange(NW))
        rank = np.arange(len(s)) - grp_first[w]
        q = (rank * gsize[w]) // counts[c][w]      # quantile-spread in window
        slot = tb_arr[w * 32 + (q // 128)] + (q % 128)
        assert (slot >= 0).all()
        pr_all[c][slot] = _perm_row(s)
        off_all[c][slot] = d & 127
        offa = off_all[c].reshape(n_chunks, TPC, 128).transpose(2, 0, 1).reshape(128, -1)
        dcol_h[c] = offa.astype(NPF16)
        drow_h[c] = off_all[c].reshape(1, -1).astype(NPBF16)

    # per-chunk gather calls with dynamic base (split if span > 31 blocks)
    real = off_all >= 0
    blk = np.where(real, pr_all // 1024, 1 << 30)
    blk_hi = np.where(real, pr_all // 1024, -1)
    gathers = []
    slot_base = np.zeros(total_slots, np.int64)
    for cidx in range(n_chunks):
        calls = []
        j = 0
        nlive = sum(1 for jj in range(TPC) if tile_w[cidx * TPC + jj] >= 0)
        while j < nlive:
            j0 = j
            s0 = cidx * CHUNK + j0 * 128
            lo = int(blk[:, s0:s0 + 128].min())
            hi = int(blk_hi[:, s0:s0 + 128].max())
            j += 1
            while j < nlive:
                s1 = cidx * CHUNK + j * 128
                nlo = min(lo, int(blk[:, s1:s1 + 128].min()))
                nhi = max(hi, int(blk_hi[:, s1:s1 + 128].max()))
                if nhi - nlo > 31:
                    break
                lo, hi = nlo, nhi
                j += 1
            if lo >= (1 << 30):
                lo = 0
            base = lo * 1024
            calls.append((j0, j - j0, int(base)))
            slot_base[cidx * CHUNK + j0 * 128: cidx * CHUNK + j * 128] = base
        if not calls:
            calls.append((0, TPC, 0))
        gathers.append(calls)

    for c in range(NCORES):
        gi = pr_all[c] - slot_base
        gi[~real[c]] = 0
        assert (gi >= 0).all() and (gi < 32768).all()
        gia = gi.reshape(n_chunks, CHUNK // 16, 16).transpose(0, 2, 1)
        idx_h[c] = np.tile(gia, (1, 8, 1)).transpose(1, 0, 2).reshape(128, -1)

    sched = dict(n_chunks=n_chunks, tile_w=tile_w.tolist(),
                 tile_first=tile_first.tolist(), tile_last=tile_last.tolist(),
                 tile_final=tile_final.tolist(), gathers=gathers)
    return sched, idx_h, dcol_h, drow_h


MARKS = []


def build(sched, debug=False, no_collective=False, reps=1):
    MARKS.clear()
    NW, NPAD, NBUCK, TBL_ROWS = _derived()
    n_chunks = sched["n_chunks"]
    tile_w = sched["tile_w"]
    tile_first = sched["tile_first"]
    tile_last = sched["tile_last"]
    tile_final = sched["tile_final"]
    gathers = sched["gathers"]
    NT_DENSE = NPAD // 128
    NB_DENSE = (NT_DENSE + 7) // 8

    nc = bacc.Bacc(None, target_bir_lowering=False, num_swdge_queues=4)

    embT = nc.dram_tensor("embT", [D, NPAD], BF16, kind="ExternalInput")
    embTo = nc.dram_tensor("embTo", [D, OWN], BF16, kind="ExternalInput")
    w1aux = nc.dram_tensor("w1aux", [D, D + H1], BF16, kind="ExternalInput")
    w1ad = nc.dram_tensor("w1ad", [D, H1], BF16, kind="ExternalInput")
    w2aux = nc.dram_tensor("w2aux", [D, D + 1], BF16, kind="ExternalInput")
    w2ad = nc.dram_tensor("w2ad", [D, 1], BF16, kind="ExternalInput")
    b1t_in = nc.dram_tensor("b1t", [128, D], F32, kind="ExternalInput")
    b2t_in = nc.dram_tensor("b2t", [128, D], F32, kind="ExternalInput")
    iota_in = nc.dram_tensor("iotac", [128, 128], F16, kind="ExternalInput")
    pconst_in = nc.dram_tensor("pconst", [128, 1], F32, kind="ExternalInput")
    ident_in = nc.dram_tensor("ident", [128, 128], F32, kind="ExternalInput")
    ones_in = nc.dram_tensor("ones1", [1, 128], BF16, kind="ExternalInput")
    idx_in = nc.dram_tensor("idx16", [128, n_chunks * (CHUNK // 16)], I16, kind="ExternalInput")
    dcol_in = nc.dram_tensor("dcol", [128, n_chunks * TPC], F16, kind="ExternalInput")
    drow_in = nc.dram_tensor("drow", [1, n_chunks * CHUNK], BF16, kind="ExternalInput")
    out_own = nc.dram_tensor("out_own", [OWN, D], F32, kind="ExternalOutput")

    if debug:
        dbg_acc1 = nc.dram_tensor("dbg_acc1", [128, NW * (D + H1)], F32,
                                  kind="ExternalOutput")
        dbg_tbl = nc.dram_tensor("dbg_tbl", [TBL_ROWS, 128], F16,
                                 kind="ExternalOutput")
    table = nc.dram_tensor("table", [TBL_ROWS, 128], F16)
    ag_in = nc.dram_tensor("ag_in", [D, OWN], BF16)
    ag_out = nc.dram_tensor("ag_out", [NCORES * D, OWN], BF16, addr_space="Shared")

    with tile.TileContext(nc) as tc:
        with tc.tile_pool(name="persist", bufs=1) as pp:
            b1t = pp.tile([128, D], F32)
            b2t = pp.tile([128, D], F32)
            iotac = pp.tile([128, 128], F16)
            pconst = pp.tile([128, 1], F32)
            ident = pp.tile([128, 128], F32)
            ones1 = pp.tile([1, 128], BF16)
            w1x = pp.tile([D, D + H1], BF16)
            w1d = pp.tile([D, H1], BF16)
            w2x = pp.tile([D, D + 1], BF16)
            w2d = pp.tile([D, 1], BF16)
            idx_s = pp.tile([128, n_chunks * (CHUNK // 16)], I16)
            dcol_s = pp.tile([128, n_chunks * TPC], F16)
            adw = pp.tile([128, NW * H1], F16)
            adw2 = pp.tile([128, NW], F16)
            acc1 = pp.tile([128, NW * (D + H1)], F32)
            acc2 = pp.tile([128, NW * (D + 1)], F32)
            for t_, s_ in [(b1t, b1t_in), (b2t, b2t_in), (iotac, iota_in),
                           (pconst, pconst_in), (ident, ident_in), (ones1, ones_in),
                           (w1x, w1aux), (w1d, w1ad), (w2x, w2aux), (w2d, w2ad),
                           (idx_s, idx_in), (dcol_s, dcol_in)]:
                nc.sync.dma_start(out=t_[:], in_=s_[:])
            def dense(layer, rep):
                """x @ Waux -> fp16 table rows (permuted layout)."""
                waux = w1x if layer == 1 else w2x
                ncol = D + H1 if layer == 1 else D + 1
                with tc.tile_pool(name=f"dns{layer}r{rep}", bufs=3) as dp, \
                     tc.tile_pool(name=f"dnp{layer}r{rep}", bufs=3, space="PSUM") as dpp:
                    for tb0 in range(0, NB_DENSE, 2):
                        nb = min(2, NB_DENSE - tb0)
                        lt = dp.tile([D, nb * 1024], BF16, tag="lhs")
                        if layer == 1:
                            nc.sync.dma_start(
                                out=lt[:], in_=embT[:, tb0 * 1024:(tb0 + nb) * 1024])
                        else:
                            # global tiles -> (core, window) runs
                            j = 0
                            while j < 8 * nb:
                                t = tb0 * 8 + j
                                co, wl = divmod(t, NW)
                                nrun = min(8 * nb - j, NW - wl)
                                nc.sync.dma_start(
                                    out=lt[:, j * 128:(j + nrun) * 128],
                                    in_=ag_out[co * D:(co + 1) * D,
                                               wl * 128:(wl + nrun) * 128])
                                j += nrun
                        stg = dp.tile([128, nb * 1024], F16, tag="stg")
                        for j in range(8 * nb):
                            ps = dpp.tile([128, ncol], F32, tag="d")
                            nc.tensor.matmul(out=ps[:], lhsT=lt[:, j * 128:(j + 1) * 128],
                                             rhs=waux[:], start=True, stop=True)
                            if j % 2 == 0:
                                nc.scalar.activation(
                                    out=stg[:, j * 128:j * 128 + ncol], in_=ps[:],
                                    func=mybir.ActivationFunctionType.Copy)
                            else:
                                nc.vector.tensor_copy(
                                    out=stg[:, j * 128:j * 128 + ncol], in_=ps[:])
                        for b in range(nb):
                            nc.sync.dma_start(
                                out=table[(tb0 + b) * 1024:(tb0 + b + 1) * 1024]
                                .rearrange("(p j) k -> p (j k)", j=8),
                                in_=stg[:, b * 1024:(b + 1) * 1024])

            def adw_fill(layer, rep):
                """Per-owned-window a_dst.h via x_own @ (W @ Ad), plus the
                dense self-loop contribution (no gather needed): h_own from
                lt @ Waux, e_self = asrc.h + adst.h, acc += [w*h | w]."""
                wad = w1d if layer == 1 else w2d
                waux = w1x if layer == 1 else w2x
                H = H1 if layer == 1 else 1
                CH = C1 if layer == 1 else D
                EC = D + H
                acc = acc1 if layer == 1 else acc2
                dst_t = adw if layer == 1 else adw2
                srcT = embTo if layer == 1 else ag_in
                with tc.tile_pool(name=f"aw{layer}r{rep}", bufs=3) as ap, \
                     tc.tile_pool(name=f"awp{layer}r{rep}", bufs=3, space="PSUM") as app:
                    ltb = None
                    for w in range(NW):
                        if w % 8 == 0:
                            nwb = min(8, NW - w)
                            ltb = ap.tile([D, 8 * 128], BF16, tag="lb")
                            nc.sync.dma_start(
                                out=ltb[:, 0:nwb * 128],
                                in_=srcT[:, w * 128:(w + nwb) * 128])
                        lt = ltb[:, (w % 8) * 128:(w % 8 + 1) * 128]
                        ps = app.tile([128, H], F32, tag="p")
                        nc.tensor.matmul(out=ps[:], lhsT=lt, rhs=wad[:],
                                         start=True, stop=True)
                        nc.scalar.activation(out=dst_t[:, w * H:(w + 1) * H], in_=ps[:],
                                             func=mybir.ActivationFunctionType.Copy)
                        psh = app.tile([128, EC], F32, tag="h")
                        nc.tensor.matmul(out=psh[:], lhsT=lt, rhs=waux[:],
                                         start=True, stop=True)
                        ho = ap.tile([128, EC], F16, tag="h16")
                        nc.scalar.activation(out=ho[:], in_=psh[:],
                                             func=mybir.ActivationFunctionType.Copy)
                        es = ap.tile([128, H], F32, tag="es")
                        nc.vector.tensor_tensor(out=es[:], in0=ho[:, D:D + H],
                                                in1=dst_t[:, w * H:(w + 1) * H],
                                                op=mybir.AluOpType.add)
                        lrs = ap.tile([128, H], F32, tag="lrs")
                        nc.vector.tensor_scalar_mul(out=lrs[:], in0=es[:], scalar1=NEG)
                        nc.vector.tensor_tensor(out=lrs[:], in0=lrs[:], in1=es[:],
                                                op=mybir.AluOpType.max)
                        wx = ap.tile([128, D], F16, tag="wx")
                        wx3 = wx[:].rearrange("p (h k) -> p h k", k=CH)
                        nc.scalar.activation(
                            out=wx3,
                            in_=lrs[:, :, None].to_broadcast([128, H, CH]),
                            func=mybir.ActivationFunctionType.Exp)
                        ms = ap.tile([128, D], F32, tag="ms")
                        nc.vector.tensor_tensor(out=ms[:], in0=ho[:, 0:D], in1=wx[:],
                                                op=mybir.AluOpType.mult)
                        nc.vector.tensor_tensor(
                            out=acc[:, w * EC:w * EC + D],
                            in0=acc[:, w * EC:w * EC + D], in1=ms[:],
                            op=mybir.AluOpType.add)
                        nc.vector.tensor_tensor(
                            out=acc[:, w * EC + D:(w + 1) * EC],
                            in0=acc[:, w * EC + D:(w + 1) * EC],
                            in1=wx3[:, :, 0],
                            op=mybir.AluOpType.add)

            def fin1(w, fp, fpp):
                """Finalize window w of layer 1: softmax div, bias, ELU,
                transpose, write ag_in column block."""
                EC = D + H1
                den = fp.tile([128, H1], F32, tag="den")
                nc.vector.tensor_scalar_add(
                    out=den[:], in0=acc1[:, w * EC + D:(w + 1) * EC], scalar1=1e-16)
                rec = fp.tile([128, H1], F32, tag="rec")
                nc.vector.reciprocal(out=rec[:], in_=den[:])
                x2 = fp.tile([128, D], F32, tag="x2")
                nc.vector.tensor_tensor(
                    out=x2[:].rearrange("p (h k) -> p h k", k=C1),
                    in0=acc1[:, w * EC:w * EC + D].rearrange("p (h k) -> p h k", k=C1),
                    in1=rec[:, :, None].to_broadcast([128, H1, C1]),
                    op=mybir.AluOpType.mult)
                nc.vector.tensor_tensor(out=x2[:], in0=x2[:], in1=b1t[:],
                                        op=mybir.AluOpType.add)
                # elu(x) = relu(x) - relu(1 - exp(x))
                ex = fp.tile([128, D], F32, tag="ex")
                nc.scalar.activation(out=ex[:], in_=x2[:],
                                     func=mybir.ActivationFunctionType.Exp)
                u = fp.tile([128, D], F32, tag="u")
                nc.scalar.activation(out=u[:], in_=ex[:],
                                     func=mybir.ActivationFunctionType.Relu,
                                     scale=-1.0, bias=1.0)
                r = fp.tile([128, D], F32, tag="r")
                nc.scalar.activation(out=r[:], in_=x2[:],
                                     func=mybir.ActivationFunctionType.Relu)
                xe = fp.tile([128, D], F32, tag="xe")
                nc.vector.tensor_tensor(out=xe[:], in0=r[:], in1=u[:],
                                        op=mybir.AluOpType.subtract)
                pst = fpp.tile([D, 128], F32, tag="t")
                nc.tensor.transpose(out=pst[:], in_=xe[:], identity=ident[:])
                xt = fp.tile([D, 128], BF16, tag="xt")
                nc.scalar.activation(out=xt[:], in_=pst[:],
                                     func=mybir.ActivationFunctionType.Copy)
                nc.sync.dma_start(out=ag_in[:, w * 128:(w + 1) * 128], in_=xt[:])

            def fin2(w, fp):
                """Finalize window w of layer 2: softmax div, bias, l2-norm,
                write out_own rows."""
                EC = D + 1
                den = fp.tile([128, 1], F32, tag="den")
                nc.vector.tensor_scalar_add(
                    out=den[:], in0=acc2[:, w * EC + D:(w + 1) * EC], scalar1=1e-16)
                rec = fp.tile([128, 1], F32, tag="rec")
                nc.vector.reciprocal(out=rec[:], in_=den[:])
                o = fp.tile([128, D], F32, tag="o")
                nc.vector.tensor_tensor(
                    out=o[:], in0=acc2[:, w * EC:w * EC + D],
                    in1=rec[:].to_broadcast([128, D]), op=mybir.AluOpType.mult)
                nc.vector.tensor_tensor(out=o[:], in0=o[:], in1=b2t[:],
                                        op=mybir.AluOpType.add)
                sq = fp.tile([128, D], F32, tag="sq")
                ss = fp.tile([128, 1], F32, tag="ss")
                nc.scalar.activation(out=sq[:], in_=o[:],
                                     func=mybir.ActivationFunctionType.Square,
                                     accum_out=ss[:])
                nrm = fp.tile([128, 1], F32, tag="nr")
                nc.scalar.activation(out=nrm[:], in_=ss[:],
                                     func=mybir.ActivationFunctionType.Sqrt)
                nc.vector.tensor_scalar_max(out=nrm[:], in0=nrm[:], scalar1=1e-12)
                rn = fp.tile([128, 1], F32, tag="rn")
                nc.vector.reciprocal(out=rn[:], in_=nrm[:])
                of = fp.tile([128, D], F32, tag="of")
                nc.vector.tensor_tensor(out=of[:], in0=o[:],
                                        in1=rn[:].to_broadcast([128, D]),
                                        op=mybir.AluOpType.mult)
                nc.sync.dma_start(out=out_own[w * 128:(w + 1) * 128, :], in_=of[:])

            def edge_sweep(layer, rep):
                H = H1 if layer == 1 else 1
                CH = C1 if layer == 1 else D
                EC = D + H
                acc = acc1 if layer == 1 else acc2
                adwl = adw if layer == 1 else adw2
                with tc.tile_pool(name=f"eg{layer}r{rep}", bufs=6) as gp, \
                     tc.tile_pool(name=f"ed{layer}r{rep}", bufs=2) as drp, \
                     tc.tile_pool(name=f"er{layer}r{rep}", bufs=4) as rp, \
                     tc.tile_pool(name=f"em{layer}r{rep}", bufs=6) as mp, \
                     tc.tile_pool(name=f"fw{layer}r{rep}", bufs=3) as fwp, \
                     tc.tile_pool(name=f"epr{layer}r{rep}", bufs=1, space="PSUM") as prp, \
                     tc.tile_pool(name=f"epa{layer}r{rep}", bufs=2, space="PSUM") as pap, \
                     tc.tile_pool(name=f"fwp{layer}r{rep}", bufs=2, space="PSUM") as fpp, \
                     tc.tile_pool(name=f"epg{layer}r{rep}", bufs=2, space="PSUM") as pgp:
                    group_ps = {}
                    gq = 0
                    for c in range(n_chunks):
                        live = [j for j in range(TPC) if tile_w[c * TPC + j] >= 0]
                        assert live == list(range(len(live))), "pads must trail"
                        nl = len(live)
                        ght = gp.tile([128, TPC * 128], F16, tag="ght")
                        ght3g = ght[:].rearrange("p (a k) -> p a k", k=128)
                        for (j0, ntl, base) in gathers[c]:
                            hi = min(base + 32768, TBL_ROWS)
                            cb = c * (CHUNK // 16)
                            nc.gpsimd.dma_gather(
                                ght3g[:, j0:j0 + ntl, :],
                                table[base:hi, :],
                                idx_s[:, cb + j0 * 8:cb + (j0 + ntl) * 8],
                                ntl * 128, ntl * 128, 128, elem_step=128,
                                queue_num=gq % 4)
                            gq += 1
                        if not live:
                            continue
                        ght3 = ght[:].rearrange("p (a k) -> p a k", k=128)
                        # replicate dstoff row via K=1 matmuls (bf16)
                        drt = drp.tile([1, CHUNK], BF16, tag="drow")
                        nc.sync.dma_start(
                            out=drt[:], in_=drow_in[:, c * CHUNK:(c + 1) * CHUNK])
                        psr16 = rp.tile([128, CHUNK], F16, tag="p16")
                        psr = prp.tile([128, CHUNK], F32, tag="r")
                        for hh in range(2):
                            nc.tensor.matmul(
                                out=psr[:, hh * 512:(hh + 1) * 512],
                                lhsT=ones1[:],
                                rhs=drt[0:1, hh * 512:(hh + 1) * 512],
                                start=True, stop=True)
                        nc.scalar.activation(out=psr16[:], in_=psr[:],
                                             func=mybir.ActivationFunctionType.Copy)
                        R = rp.tile([128, TPC * 128], F16, tag="R")
                        nc.vector.tensor_tensor(
                            out=R[:].rearrange("p (a k) -> p a k", k=128),
                            in0=dcol_s[:, c * TPC:(c + 1) * TPC, None].to_broadcast(
                                [128, TPC, 128]),
                            in1=iotac[:, None, :].to_broadcast([128, TPC, 128]),
                            op=mybir.AluOpType.is_equal)
                        RT = rp.tile([128, TPC * 128], F16, tag="RT")
                        nc.vector.tensor_scalar(
                            out=RT[:], in0=psr16[:], scalar1=pconst[:],
                            scalar2=None, op0=mybir.AluOpType.is_equal)
                        psa = pap.tile([128, nl * H], F32, tag="a", name=f"psa{c}")
                        for j in live:
                            w = tile_w[c * TPC + j]
                            nc.tensor.matmul(
                                out=psa[:, j * H:(j + 1) * H],
                                lhsT=RT[:, j * 128:(j + 1) * 128],
                                rhs=adwl[:, w * H:(w + 1) * H],
                                start=True, stop=True)
                        ew = mp.tile([128, nl * H], F32, tag="ew", name=f"ew{c}")
                        nc.vector.tensor_tensor(
                            out=ew[:].rearrange("p (a h) -> p a h", h=H),
                            in0=psa[:].rearrange("p (a h) -> p a h", h=H),
                            in1=ght3[:, 0:nl, D:D + H],
                            op=mybir.AluOpType.add)
                        lr = mp.tile([128, nl * H], F32, tag="lr", name=f"lr{c}")
                        nc.vector.tensor_scalar_mul(out=lr[:], in0=ew[:], scalar1=NEG)
                        nc.vector.tensor_tensor(out=lr[:], in0=lr[:], in1=ew[:],
                                                op=mybir.AluOpType.max)
                        # exp, pre-expanded across the C dim (Act) -> 2x DVE mult
                        we16 = mp.tile([128, nl * D], F16, tag="we", name=f"we{c}")
                        we4 = we16[:].rearrange("p (a h k) -> p a h k", h=H, k=CH)
                        nc.scalar.activation(
                            out=we4,
                            in_=lr[:].rearrange("p (a h) -> p a h", h=H)[:, :, :, None]
                                .to_broadcast([128, nl, H, CH]),
                            func=mybir.ActivationFunctionType.Exp)
                        msgt = mp.tile([128, nl * EC], F16, tag="msg", name=f"msg{c}")
                        msgt3 = msgt[:].rearrange("p (a k) -> p a k", k=EC)
                        nc.vector.tensor_copy(out=msgt3[:, :, D:D + H],
                                              in_=we4[:, :, :, 0])
                        nc.vector.tensor_tensor(
                            out=msgt3[:, :, 0:D].rearrange("p a (h k) -> p a h k", k=CH),
                            in0=ght3[:, 0:nl, 0:D].rearrange("p a (h k) -> p a h k", k=CH),
                            in1=we4,
                            op=mybir.AluOpType.mult)
                        for j in live:
                            t = c * TPC + j
                            w = tile_w[t]
                            if tile_first[t]:
                                group_ps[w] = pgp.tile([128, EC], F32, tag="g", name=f"grp{w}")
                            ps = group_ps[w]
                            nc.tensor.matmul(
                                out=ps[:], lhsT=R[:, j * 128:(j + 1) * 128],
                                rhs=msgt[:, j * EC:(j + 1) * EC],
                                start=tile_first[t], stop=tile_last[t])
                            if tile_last[t]:
                                nc.vector.tensor_tensor(
                                    out=acc[:, w * EC:(w + 1) * EC],
                                    in0=acc[:, w * EC:(w + 1) * EC],
                                    in1=ps[:], op=mybir.AluOpType.add)
                                del group_ps[w]
                                if tile_final[t]:
                                    if layer == 1:
                                        fin1(w, fwp, fpp)
                                    else:
                                        fin2(w, fwp)
                    assert not group_ps

            for rep in range(reps):
                nc.vector.memset(acc1[:], 0.0)
                nc.vector.memset(acc2[:], 0.0)
                # ================= layer 1 =================
                if rep == 0:
                        MARKS.append(("adw1", nc.next_id()))
                adw_fill(1, rep)
                if rep == 0:
                        MARKS.append(("dense1", nc.next_id()))
                dense(1, rep)
                if debug:
                    with tc.tile_pool(name="dbgt", bufs=2) as dtp:
                        for tb in range(NB_DENSE):
                            t_ = dtp.tile([128, 8 * 128], F16, tag="d")
                            nc.sync.dma_start(
                                out=t_[:],
                                in_=table[tb * 1024:(tb + 1) * 1024].rearrange(
                                    "(p j) k -> p (j k)", j=8))
                            nc.sync.dma_start(
                                out=dbg_tbl[tb * 1024:(tb + 1) * 1024].rearrange(
                                    "(p j) k -> p (j k)", j=8),
                                in_=t_[:])
                if rep == 0:
                        MARKS.append(("sweep1", nc.next_id()))
                edge_sweep(1, rep)
                if debug:
                    nc.sync.dma_start(out=dbg_acc1[:], in_=acc1[:])
                if rep == 0:
                        MARKS.append(("collective", nc.next_id()))
                if no_collective:
                    with tc.tile_pool(name=f"agcr{rep}", bufs=2) as acp:
                        for cc in range(NCORES):
                            t_ = acp.tile([D, OWN], BF16, tag="agc")
                            nc.sync.dma_start(out=t_[:], in_=ag_in[:])
                            nc.sync.dma_start(out=ag_out[cc * D:(cc + 1) * D, :], in_=t_[:])
                else:
                    nc.gpsimd.collective_compute(
                        "AllGather", mybir.AluOpType.bypass,
                        ins=[ag_in[:]], outs=[ag_out[:]],
                        replica_groups=[list(range(NCORES))])

                # ================= layer 2 =================
                if rep == 0:
                        MARKS.append(("adw2", nc.next_id()))
                adw_fill(2, rep)
                if rep == 0:
                        MARKS.append(("dense2", nc.next_id()))
                dense(2, rep)
                if rep == 0:
                        MARKS.append(("sweep2", nc.next_id()))
                edge_sweep(2, rep)
    return nc


def make_inputs(edge_index, emb, W1, a_src1, a_dst1, b1, W2, a_src2, a_dst2, b2):
    NW, NPAD, NBUCK, TBL_ROWS = _derived()
    sched, idx_h, dcol_h, drow_h = prep(edge_index)

    W1 = np.asarray(W1, np.float32)
    a_s1 = np.asarray(a_src1, np.float32)
    a_d1 = np.asarray(a_dst1, np.float32)
    As = np.zeros((D, H1), np.float32)
    Ad = np.zeros((D, H1), np.float32)
    for h in range(H1):
        As[h * C1:(h + 1) * C1, h] = a_s1[h]
        Ad[h * C1:(h + 1) * C1, h] = a_d1[h]
    w1x = np.concatenate([W1, W1 @ As], 1).astype(NPBF16)
    w1d = (W1 @ Ad).astype(NPBF16)
    W2 = np.asarray(W2, np.float32)
    w2x = np.concatenate([W2, W2 @ np.asarray(a_src2, np.float32).T], 1).astype(NPBF16)
    w2d = (W2 @ np.asarray(a_dst2, np.float32).T).astype(NPBF16)

    embT = np.zeros((D, NPAD), NPBF16)
    embT[:, :N] = np.asarray(emb, np.float32).T.astype(NPBF16)
    iotac = np.broadcast_to(np.arange(128, dtype=NPF16)[None, :], (128, 128)).copy()
    pconst = np.arange(128, dtype=np.float32)[:, None].copy()
    ident = np.eye(128, dtype=np.float32)
    ones1 = np.ones((1, 128), NPBF16)
    b1t = np.broadcast_to(np.asarray(b1, np.float32)[None, :], (128, D)).copy()
    b2t = np.broadcast_to(np.asarray(b2, np.float32)[None, :], (128, D)).copy()

    in_maps = []
    for c in range(NCORES):
        in_maps.append({
            "embT": embT, "embTo": np.ascontiguousarray(embT[:, c * OWN:(c + 1) * OWN]),
            "w1aux": w1x, "w1ad": w1d, "w2aux": w2x, "w2ad": w2d,
            "b1t": b1t, "b2t": b2t, "iotac": iotac, "pconst": pconst,
            "ident": ident, "ones1": ones1,
            "idx16": idx_h[c], "dcol": dcol_h[c], "drow": drow_h[c],
        })
    return sched, in_maps


def kernel(edge_index, emb, W1, a_src1, a_dst1, b1, W2, a_src2, a_dst2, b2):
    sched, in_maps = make_inputs(edge_index, emb, W1, a_src1, a_dst1, b1,
                                 W2, a_src2, a_dst2, b2)
    nc = build(sched)
    nc.finalize()
    res = run_bass_kernel_spmd(nc, in_maps, core_ids=list(range(NCORES)))
    out = np.zeros((N, D), np.float32)
    for c in range(NCORES):
        lo, hi = c * OWN, min((c + 1) * OWN, N)
        if lo < N:
            out[lo:hi] = res.results[c]["out_own"][:hi - lo]
    return out



# revision 3
# speedup vs baseline: 1.2542x; 1.2542x over previous
"""Bass/Trainium2 kernel for the 2-layer GAT (nn_GAT_11106785427688).

Strategy (8 NeuronCores, SPMD single NEFF):
- dst-ownership sharding: core c owns nodes [c*OWN, (c+1)*OWN); it receives
  every edge whose dst it owns (~137K edges), so segment-softmax denominators
  and message sums complete locally -- no all-reduce. One AllGather of the
  layer-1 activations between layers; host assembles the final output from
  per-core slices.
- Per-edge gather of packed [h | a_src.h] rows (fp16, 256B) from an HBM table
  via the SWDGE dma_gather custom op (int16 indices -> src buckets of 32768
  rows; table rows permuted so the dense phase writes 2KB-contiguous runs).
- No indexed scatter (HW dma_scatter_add loses duplicate updates): edges are
  grouped by 128-node dst window; the one-hot R [edges x nodes] and its
  transpose RT [nodes x edges] are PRECOMPUTED ON HOST (pure edge-index
  preprocessing) and streamed from HBM, so the DVE never builds one-hots.
  R turns segment-sum into PE matmul accumulated in PSUM; RT gathers the
  per-window a_dst values to edges via PE. Softmax division is deferred:
  out = (sum_e w*h[src]) / (sum_e w).
- exp(leakyrelu(e)) computed without max-subtraction (shift-invariant).
- adw_fill (self-loops + per-window a_dst) is interleaved with the dense
  table build so PE/Act/DVE/DMA overlap instead of running serial phases.
"""
import numpy as np
import ml_dtypes

from concourse import bacc, mybir
import concourse.tile as tile
from concourse.bass_utils import run_bass_kernel_spmd

# ---- problem constants ----
N = 100000
D = 64
H1, C1 = 4, 16
NEG = 0.2
NCORES = 8
OWN = 12544                 # 98 windows * 128 per core
BUCK = 32768
CHUNK = 1024                # gather idxs per dma_gather call (ring limit)
TPC = CHUNK // 128          # tiles per chunk = 8

F16 = mybir.dt.float16
F32 = mybir.dt.float32
BF16 = mybir.dt.bfloat16
I16 = mybir.dt.int16
NPF16 = np.float16
NPBF16 = ml_dtypes.bfloat16

ACT = mybir.ActivationFunctionType


def _derived():
    NW = OWN // 128
    NPAD = NCORES * OWN
    NBUCK = (NPAD + BUCK - 1) // BUCK
    TBL_ROWS = NBUCK * BUCK
    return NW, NPAD, NBUCK, TBL_ROWS


def _perm_row(src):
    """Permuted table row for node src: tb*1024 + p*8 + j (write-friendly)."""
    tb, r = np.divmod(src, 1024)
    j, p = np.divmod(r, 128)
    return tb * 1024 + p * 8 + j


def prep(edge_index):
    """Vectorized host prep: quantile-banded schedule.

    Per-(core,window) edges sorted by src, quantile-spread into the padded
    window group (G_w = roundup128(max-over-cores)). Window tiles are split
    into bands of <=3 tiles; the schedule is band-major so consecutive tiles
    cover the same src-quantile region. Each 1024-slot chunk then spans <=~31
    perm-blocks and gets ONE dma_gather call with a dynamic host-computed
    base (int16 idx). Bands are chunk-aligned (pad tiles trail per band).

    Also builds, per core, the fp16 one-hot streams R (edge-major: used as
    matmul lhsT for the per-window segment sums) and RT (node-major: used as
    lhsT to gather per-window a_dst values to edge positions).
    """
    NW, NPAD, NBUCK, TBL_ROWS = _derived()
    # self-loops are handled densely in adw_fill, not in the gather sweep
    src = np.asarray(edge_index[0])
    dst = np.asarray(edge_index[1])
    owner = dst // OWN

    per_core = []
    counts = np.zeros((NCORES, NW), np.int64)
    for c in range(NCORES):
        m = owner == c
        s = src[m]
        d = dst[m] - c * OWN
        w = d >> 7
        order = np.lexsort((s, w))
        s, d, w = s[order], d[order], w[order]
        per_core.append((s, d, w))
        counts[c] = np.bincount(w, minlength=NW)

    gsize = ((counts.max(0) + 127) // 128 * 128).astype(np.int64)   # [NW]
    kw = gsize // 128                                               # tiles/window

    # band-major tile schedule: band b = quantile quarter [b/4,(b+1)/4) of
    # every window, so run centers align across windows regardless of K_w
    NBANDS = 4
    kb = [[int(round(b * int(kw[w]) / NBANDS)) for b in range(NBANDS + 1)]
          for w in range(NW)]
    tile_list = []          # (w, k) in schedule order
    for b in range(NBANDS):
        for w in range(NW):
            for k in range(kb[w][b], kb[w][b + 1]):
                tile_list.append((w, k))
        # chunk-align each band (pad tiles trail inside the band's last chunk)
        while len(tile_list) % TPC != 0:
            tile_list.append((-1, -1))

    n_tiles = len(tile_list)
    total_slots = n_tiles * 128
    n_chunks = total_slots // CHUNK
    tile_w = np.array([w for w, _ in tile_list], np.int64)
    # slot base of each (w,k) tile
    tile_base = {}
    for t, (w, k) in enumerate(tile_list):
        if w >= 0:
            tile_base[(w, k)] = t * 128
    # first/last per (window, band) run
    tile_first = np.zeros(n_tiles, bool)
    tile_last = np.zeros(n_tiles, bool)
    tile_final = np.zeros(n_tiles, bool)
    for t, (w, k) in enumerate(tile_list):
        if w < 0:
            continue
        tile_first[t] = k in [kb[w][b] for b in range(NBANDS)]
        tile_last[t] = (k + 1) in [kb[w][b + 1] for b in range(NBANDS)]
        tile_final[t] = k + 1 == int(kw[w])

    # per-core slot arrays + per-tile block ranges
    idx_h = np.zeros((NCORES, 128, n_chunks * (CHUNK // 16)), np.int16)
    R_h = np.zeros((NCORES, 128, n_tiles * 128), NPF16)
    RT_h = np.zeros((NCORES, 128, n_tiles * 128), NPF16)
    pr_all = np.zeros((NCORES, total_slots), np.int64)
    off_all = np.full((NCORES, total_slots), -1, np.int64)
    tb_arr = np.full(NW * 32, -1, np.int64)
    for (w, k), sb in tile_base.items():
        tb_arr[w * 32 + k] = sb
    kidx = np.arange(128)
    for c in range(NCORES):
        s, d, w = per_core[c]
        grp_first = np.searchsorted(w, np.arange(NW))
        rank = np.arange(len(s)) - grp_first[w]
        q = (rank * gsize[w]) // counts[c][w]      # quantile-spread in window
        slot = tb_arr[w * 32 + (q // 128)] + (q % 128)
        assert (slot >= 0).all()
        pr_all[c][slot] = _perm_row(s)
        off_all[c][slot] = d & 127
        offs = off_all[c].reshape(n_tiles, 128)
        # R[p, t*128+k] = (off(slot t*128+p) == k); pads (off=-1) -> zero col
        R_h[c] = (offs[:, :, None] == kidx[None, None, :]) \
            .transpose(1, 0, 2).reshape(128, -1).astype(NPF16)
        # RT[p, t*128+e] = (off(slot t*128+e) == p)
        RT_h[c] = (offs[None, :, :] == kidx[:, None, None]) \
            .reshape(128, -1).astype(NPF16)

    # per-chunk gather calls with dynamic base (split if span > 31 blocks)
    real = off_all >= 0
    blk = np.where(real, pr_all // 1024, 1 << 30)
    blk_hi = np.where(real, pr_all // 1024, -1)
    gathers = []
    slot_base = np.zeros(total_slots, np.int64)
    for cidx in range(n_chunks):
        calls = []
        j = 0
        nlive = sum(1 for jj in range(TPC) if tile_w[cidx * TPC + jj] >= 0)
        while j < nlive:
            j0 = j
            s0 = cidx * CHUNK + j0 * 128
            lo = int(blk[:, s0:s0 + 128].min())
            hi = int(blk_hi[:, s0:s0 + 128].max())
            j += 1
            while j < nlive:
                s1 = cidx * CHUNK + j * 128
                nlo = min(lo, int(blk[:, s1:s1 + 128].min()))
                nhi = max(hi, int(blk_hi[:, s1:s1 + 128].max()))
                if nhi - nlo > 31:
                    break
                lo, hi = nlo, nhi
                j += 1
            if lo >= (1 << 30):
                lo = 0
            base = lo * 1024
            calls.append((j0, j - j0, int(base)))
            slot_base[cidx * CHUNK + j0 * 128: cidx * CHUNK + j * 128] = base
        if not calls:
            calls.append((0, TPC, 0))
        gathers.append(calls)

    for c in range(NCORES):
        gi = pr_all[c] - slot_base
        gi[~real[c]] = 0
        assert (gi >= 0).all() and (gi < 32768).all()
        gia = gi.reshape(n_chunks, CHUNK // 16, 16).transpose(0, 2, 1)
        idx_h[c] = np.tile(gia, (1, 8, 1)).transpose(1, 0, 2).reshape(128, -1)

    sched = dict(n_chunks=n_chunks, tile_w=tile_w.tolist(),
                 tile_first=tile_first.tolist(), tile_last=tile_last.tolist(),
                 tile_final=tile_final.tolist(), gathers=gathers)
    return sched, idx_h, R_h, RT_h


MARKS = []


def build(sched, debug=False, no_collective=False, reps=1):
    MARKS.clear()
    NW, NPAD, NBUCK, TBL_ROWS = _derived()
    n_chunks = sched["n_chunks"]
    tile_w = sched["tile_w"]
    tile_first = sched["tile_first"]
    tile_last = sched["tile_last"]
    tile_final = sched["tile_final"]
    gathers = sched["gathers"]
    n_tiles = n_chunks * TPC
    NT_DENSE = NPAD // 128
    NB_DENSE = (NT_DENSE + 7) // 8

    nc = bacc.Bacc(None, target_bir_lowering=False, num_swdge_queues=4)

    embT = nc.dram_tensor("embT", [D, NPAD], BF16, kind="ExternalInput")
    embTo = nc.dram_tensor("embTo", [D, OWN], BF16, kind="ExternalInput")
    w1aux = nc.dram_tensor("w1aux", [D, D + H1], BF16, kind="ExternalInput")
    w1ad = nc.dram_tensor("w1ad", [D, H1], BF16, kind="ExternalInput")
    w2aux = nc.dram_tensor("w2aux", [D, D + 1], BF16, kind="ExternalInput")
    w2ad = nc.dram_tensor("w2ad", [D, 1], BF16, kind="ExternalInput")
    b1t_in = nc.dram_tensor("b1t", [128, D], F32, kind="ExternalInput")
    b2t_in = nc.dram_tensor("b2t", [128, D], F32, kind="ExternalInput")
    ident_in = nc.dram_tensor("ident", [128, 128], F32, kind="ExternalInput")
    idx_in = nc.dram_tensor("idx16", [128, n_chunks * (CHUNK // 16)], I16, kind="ExternalInput")
    R_in = nc.dram_tensor("Rh", [128, n_tiles * 128], F16, kind="ExternalInput")
    RT_in = nc.dram_tensor("RTh", [128, n_tiles * 128], F16, kind="ExternalInput")
    out_own = nc.dram_tensor("out_own", [OWN, D], F32, kind="ExternalOutput")

    table = nc.dram_tensor("table", [TBL_ROWS, 128], F16)
    ag_in = nc.dram_tensor("ag_in", [D, OWN], BF16)
    ag_out = nc.dram_tensor("ag_out", [NCORES * D, OWN], BF16, addr_space="Shared")

    with tile.TileContext(nc) as tc:
        with tc.tile_pool(name="persist", bufs=1) as pp:
            b1t = pp.tile([128, D], F32)
            b2t = pp.tile([128, D], F32)
            ident = pp.tile([128, 128], F32)
            w1x = pp.tile([D, D + H1], BF16)
            w1d = pp.tile([D, H1], BF16)
            w2x = pp.tile([D, D + 1], BF16)
            w2d = pp.tile([D, 1], BF16)
            idx_s = pp.tile([128, n_chunks * (CHUNK // 16)], I16)
            adw = pp.tile([128, NW * H1], F16)
            adw2 = pp.tile([128, NW], F16)
            acc1 = pp.tile([128, NW * (D + H1)], F32)
            acc2 = pp.tile([128, NW * (D + 1)], F32)
            for t_, s_ in [(b1t, b1t_in), (b2t, b2t_in), (ident, ident_in),
                           (w1x, w1aux), (w1d, w1ad), (w2x, w2aux), (w2d, w2ad),
                           (idx_s, idx_in)]:
                nc.sync.dma_start(out=t_[:], in_=s_[:])

            def layer_front(layer, rep):
                """Interleaved adw_fill + dense table build.

                adw: per owned window, a_dst.h via x_own @ (W @ Ad) plus the
                dense self-loop contribution (e_self = asrc.h + adst.h,
                acc += [w*h | w]).  dense: x @ Waux -> fp16 table rows
                (permuted layout).  Emitted interleaved under shared pools so
                Tile overlaps them across engines.
                """
                wad = w1d if layer == 1 else w2d
                waux = w1x if layer == 1 else w2x
                H = H1 if layer == 1 else 1
                CH = C1 if layer == 1 else D
                EC = D + H
                ncol = EC
                acc = acc1 if layer == 1 else acc2
                dst_t = adw if layer == 1 else adw2
                srcT = embTo if layer == 1 else ag_in
                with tc.tile_pool(name=f"aw{layer}r{rep}", bufs=3) as ap, \
                     tc.tile_pool(name=f"awp{layer}r{rep}", bufs=2, space="PSUM") as app, \
                     tc.tile_pool(name=f"dns{layer}r{rep}", bufs=3) as dp, \
                     tc.tile_pool(name=f"dnp{layer}r{rep}", bufs=4, space="PSUM") as dpp:

                    def adw_step(w, ltb):
                        lt = ltb[:, (w % 8) * 128:(w % 8 + 1) * 128]
                        ps = app.tile([128, H], F32, tag="p")
                        nc.tensor.matmul(out=ps[:], lhsT=lt, rhs=wad[:],
                                         start=True, stop=True)
                        nc.scalar.activation(out=dst_t[:, w * H:(w + 1) * H], in_=ps[:],
                                             func=ACT.Copy)
                        psh = app.tile([128, EC], F32, tag="h")
                        nc.tensor.matmul(out=psh[:], lhsT=lt, rhs=waux[:],
                                         start=True, stop=True)
                        ho = ap.tile([128, EC], F16, tag="h16")
                        nc.scalar.activation(out=ho[:], in_=psh[:], func=ACT.Copy)
                        es = ap.tile([128, H], F32, tag="es")
                        nc.vector.tensor_tensor(out=es[:], in0=ho[:, D:D + H],
                                                in1=dst_t[:, w * H:(w + 1) * H],
                                                op=mybir.AluOpType.add)
                        lrs = ap.tile([128, H], F32, tag="lrs")
                        nc.scalar.activation(out=lrs[:], in_=es[:],
                                             func=ACT.Lrelu, alpha=NEG)
                        wx = ap.tile([128, D], F16, tag="wx")
                        wx3 = wx[:].rearrange("p (h k) -> p h k", k=CH)
                        nc.scalar.activation(
                            out=wx3,
                            in_=lrs[:, :, None].to_broadcast([128, H, CH]),
                            func=ACT.Exp)
                        ms = ap.tile([128, D], F32, tag="ms")
                        nc.vector.tensor_tensor(out=ms[:], in0=ho[:, 0:D], in1=wx[:],
                                                op=mybir.AluOpType.mult)
                        nc.vector.tensor_tensor(
                            out=acc[:, w * EC:w * EC + D],
                            in0=acc[:, w * EC:w * EC + D], in1=ms[:],
                            op=mybir.AluOpType.add)
                        nc.vector.tensor_tensor(
                            out=acc[:, w * EC + D:(w + 1) * EC],
                            in0=acc[:, w * EC + D:(w + 1) * EC],
                            in1=wx3[:, :, 0],
                            op=mybir.AluOpType.add)

                    def dense_step(tb0):
                        nb = min(2, NB_DENSE - tb0)
                        lt = dp.tile([D, 2 * 1024], BF16, tag="lhs")
                        if layer == 1:
                            nc.sync.dma_start(
                                out=lt[:, 0:nb * 1024],
                                in_=embT[:, tb0 * 1024:(tb0 + nb) * 1024])
                        else:
                            # global tiles -> (core, window) runs
                            j = 0
                            while j < 8 * nb:
                                t = tb0 * 8 + j
                                co, wl = divmod(t, NW)
                                nrun = min(8 * nb - j, NW - wl)
                                nc.sync.dma_start(
                                    out=lt[:, j * 128:(j + nrun) * 128],
                                    in_=ag_out[co * D:(co + 1) * D,
                                               wl * 128:(wl + nrun) * 128])
                                j += nrun
                        stg = dp.tile([128, 2 * 1024], F16, tag="stg")
                        for j in range(8 * nb):
                            ps = dpp.tile([128, ncol], F32, tag="d")
                            nc.tensor.matmul(out=ps[:], lhsT=lt[:, j * 128:(j + 1) * 128],
                                             rhs=waux[:], start=True, stop=True)
                            if j % 2 == 0:
                                nc.scalar.activation(
                                    out=stg[:, j * 128:j * 128 + ncol], in_=ps[:],
                                    func=ACT.Copy)
                            else:
                                nc.vector.tensor_copy(
                                    out=stg[:, j * 128:j * 128 + ncol], in_=ps[:])
                        for b in range(nb):
                            nc.sync.dma_start(
                                out=table[(tb0 + b) * 1024:(tb0 + b + 1) * 1024]
                                .rearrange("(p j) k -> p (j k)", j=8),
                                in_=stg[:, b * 1024:(b + 1) * 1024])

                    ltb = None
                    for w in range(NW):
                        if w % 8 == 0:
                            nwb = min(8, NW - w)
                            ltb = ap.tile([D, 8 * 128], BF16, tag="lb")
                            nc.sync.dma_start(
                                out=ltb[:, 0:nwb * 128],
                                in_=srcT[:, w * 128:(w + nwb) * 128])
                        adw_step(w, ltb)
                        if w % 2 == 0 and w < NB_DENSE:
                            dense_step(2 * (w // 2))

            def fin1(w, fp, fpp):
                """Finalize window w of layer 1: softmax div, bias, ELU,
                transpose, write ag_in column block."""
                EC = D + H1
                den = fp.tile([128, H1], F32, tag="den")
                nc.vector.tensor_scalar_add(
                    out=den[:], in0=acc1[:, w * EC + D:(w + 1) * EC], scalar1=1e-16)
                rec = fp.tile([128, H1], F32, tag="rec")
                nc.vector.reciprocal(out=rec[:], in_=den[:])
                x2 = fp.tile([128, D], F32, tag="x2")
                nc.vector.tensor_tensor(
                    out=x2[:].rearrange("p (h k) -> p h k", k=C1),
                    in0=acc1[:, w * EC:w * EC + D].rearrange("p (h k) -> p h k", k=C1),
                    in1=rec[:, :, None].to_broadcast([128, H1, C1]),
                    op=mybir.AluOpType.mult)
                nc.vector.tensor_tensor(out=x2[:], in0=x2[:], in1=b1t[:],
                                        op=mybir.AluOpType.add)
                # elu(x) = relu(x) - relu(1 - exp(x))
                ex = fp.tile([128, D], F32, tag="ex")
                nc.scalar.activation(out=ex[:], in_=x2[:], func=ACT.Exp)
                u = fp.tile([128, D], F32, tag="u")
                nc.scalar.activation(out=u[:], in_=ex[:], func=ACT.Relu,
                                     scale=-1.0, bias=1.0)
                r = fp.tile([128, D], F32, tag="r")
                nc.scalar.activation(out=r[:], in_=x2[:], func=ACT.Relu)
                xe = fp.tile([128, D], F32, tag="xe")
                nc.vector.tensor_tensor(out=xe[:], in0=r[:], in1=u[:],
                                        op=mybir.AluOpType.subtract)
                pst = fpp.tile([D, 128], F32, tag="t")
                nc.tensor.transpose(out=pst[:], in_=xe[:], identity=ident[:])
                xt = fp.tile([D, 128], BF16, tag="xt")
                nc.scalar.activation(out=xt[:], in_=pst[:], func=ACT.Copy)
                nc.sync.dma_start(out=ag_in[:, w * 128:(w + 1) * 128], in_=xt[:])

            def fin2(w, fp):
                """Finalize window w of layer 2: softmax div, bias, l2-norm,
                write out_own rows."""
                EC = D + 1
                den = fp.tile([128, 1], F32, tag="den")
                nc.vector.tensor_scalar_add(
                    out=den[:], in0=acc2[:, w * EC + D:(w + 1) * EC], scalar1=1e-16)
                rec = fp.tile([128, 1], F32, tag="rec")
                nc.vector.reciprocal(out=rec[:], in_=den[:])
                o = fp.tile([128, D], F32, tag="o")
                nc.vector.tensor_tensor(
                    out=o[:], in0=acc2[:, w * EC:w * EC + D],
                    in1=rec[:].to_broadcast([128, D]), op=mybir.AluOpType.mult)
                nc.vector.tensor_tensor(out=o[:], in0=o[:], in1=b2t[:],
                                        op=mybir.AluOpType.add)
                sq = fp.tile([128, D], F32, tag="sq")
                ss = fp.tile([128, 1], F32, tag="ss")
                nc.scalar.activation(out=sq[:], in_=o[:], func=ACT.Square,
                                     accum_out=ss[:])
                nrm = fp.tile([128, 1], F32, tag="nr")
                nc.scalar.activation(out=nrm[:], in_=ss[:], func=ACT.Sqrt)
                nc.vector.tensor_scalar_max(out=nrm[:], in0=nrm[:], scalar1=1e-12)
                rn = fp.tile([128, 1], F32, tag="rn")
                nc.vector.reciprocal(out=rn[:], in_=nrm[:])
                of = fp.tile([128, D], F32, tag="of")
                nc.vector.tensor_tensor(out=of[:], in0=o[:],
                                        in1=rn[:].to_broadcast([128, D]),
                                        op=mybir.AluOpType.mult)
                nc.sync.dma_start(out=out_own[w * 128:(w + 1) * 128, :], in_=of[:])

            def edge_sweep(layer, rep):
                H = H1 if layer == 1 else 1
                CH = C1 if layer == 1 else D
                EC = D + H
                acc = acc1 if layer == 1 else acc2
                adwl = adw if layer == 1 else adw2
                with tc.tile_pool(name=f"eg{layer}r{rep}", bufs=6) as gp, \
                     tc.tile_pool(name=f"er{layer}r{rep}", bufs=4) as rp, \
                     tc.tile_pool(name=f"em{layer}r{rep}", bufs=6) as mp, \
                     tc.tile_pool(name=f"fw{layer}r{rep}", bufs=3) as fwp, \
                     tc.tile_pool(name=f"epa{layer}r{rep}", bufs=2, space="PSUM") as pap, \
                     tc.tile_pool(name=f"fwp{layer}r{rep}", bufs=2, space="PSUM") as fpp, \
                     tc.tile_pool(name=f"epg{layer}r{rep}", bufs=2, space="PSUM") as pgp:
                    group_ps = {}
                    gq = 0
                    for c in range(n_chunks):
                        live = [j for j in range(TPC) if tile_w[c * TPC + j] >= 0]
                        assert live == list(range(len(live))), "pads must trail"
                        nl = len(live)
                        ght = gp.tile([128, TPC * 128], F16, tag="ght")
                        ght3g = ght[:].rearrange("p (a k) -> p a k", k=128)
                        for (j0, ntl, base) in gathers[c]:
                            hi = min(base + 32768, TBL_ROWS)
                            cb = c * (CHUNK // 16)
                            nc.gpsimd.dma_gather(
                                ght3g[:, j0:j0 + ntl, :],
                                table[base:hi, :],
                                idx_s[:, cb + j0 * 8:cb + (j0 + ntl) * 8],
                                ntl * 128, ntl * 128, 128, elem_step=128,
                                queue_num=gq % 4)
                            gq += 1
                        if not live:
                            continue
                        ght3 = ght[:].rearrange("p (a k) -> p a k", k=128)
                        # one-hot streams from host (R: edge-major, RT: node-major)
                        Rt = rp.tile([128, TPC * 128], F16, tag="R")
                        nc.sync.dma_start(
                            out=Rt[:, 0:nl * 128],
                            in_=R_in[:, (c * TPC) * 128:(c * TPC + nl) * 128])
                        RTt = rp.tile([128, TPC * 128], F16, tag="RT")
                        nc.sync.dma_start(
                            out=RTt[:, 0:nl * 128],
                            in_=RT_in[:, (c * TPC) * 128:(c * TPC + nl) * 128])
                        # per-edge a_dst via PE gather: psa = RT_j^T @ adw_w
                        psa = pap.tile([128, nl * H], F32, tag="a", name=f"psa{c}")
                        for j in live:
                            w = tile_w[c * TPC + j]
                            nc.tensor.matmul(
                                out=psa[:, j * H:(j + 1) * H],
                                lhsT=RTt[:, j * 128:(j + 1) * 128],
                                rhs=adwl[:, w * H:(w + 1) * H],
                                start=True, stop=True)
                        ew = mp.tile([128, nl * H], F32, tag="ew", name=f"ew{c}")
                        nc.vector.tensor_tensor(
                            out=ew[:].rearrange("p (a h) -> p a h", h=H),
                            in0=psa[:].rearrange("p (a h) -> p a h", h=H),
                            in1=ght3[:, 0:nl, D:D + H],
                            op=mybir.AluOpType.add)
                        lr = mp.tile([128, nl * H], F32, tag="lr", name=f"lr{c}")
                        nc.scalar.activation(out=lr[:], in_=ew[:],
                                             func=ACT.Lrelu, alpha=NEG)
                        # exp, pre-expanded across the C dim (Act) -> DVE mult
                        we16 = mp.tile([128, nl * D], F16, tag="we", name=f"we{c}")
                        we4 = we16[:].rearrange("p (a h k) -> p a h k", h=H, k=CH)
                        nc.scalar.activation(
                            out=we4,
                            in_=lr[:].rearrange("p (a h) -> p a h", h=H)[:, :, :, None]
                                .to_broadcast([128, nl, H, CH]),
                            func=ACT.Exp)
                        msgt = mp.tile([128, nl * EC], F16, tag="msg", name=f"msg{c}")
                        msgt3 = msgt[:].rearrange("p (a k) -> p a k", k=EC)
                        # denominator cols written directly by a second exp (Act)
                        nc.scalar.activation(
                            out=msgt3[:, :, D:D + H],
                            in_=lr[:].rearrange("p (a h) -> p a h", h=H),
                            func=ACT.Exp)
                        nc.vector.tensor_tensor(
                            out=msgt3[:, :, 0:D].rearrange("p a (h k) -> p a h k", k=CH),
                            in0=ght3[:, 0:nl, 0:D].rearrange("p a (h k) -> p a h k", k=CH),
                            in1=we4,
                            op=mybir.AluOpType.mult)
                        for j in live:
                            t = c * TPC + j
                            w = tile_w[t]
                            if tile_first[t]:
                                group_ps[w] = pgp.tile([128, EC], F32, tag="g", name=f"grp{w}")
                            ps = group_ps[w]
                            nc.tensor.matmul(
                                out=ps[:], lhsT=Rt[:, j * 128:(j + 1) * 128],
                                rhs=msgt[:, j * EC:(j + 1) * EC],
                                start=tile_first[t], stop=tile_last[t])
                            if tile_last[t]:
                                nc.vector.tensor_tensor(
                                    out=acc[:, w * EC:(w + 1) * EC],
                                    in0=acc[:, w * EC:(w + 1) * EC],
                                    in1=ps[:], op=mybir.AluOpType.add)
                                del group_ps[w]
                                if tile_final[t]:
                                    if layer == 1:
                                        fin1(w, fwp, fpp)
                                    else:
                                        fin2(w, fwp)
                    assert not group_ps

            for rep in range(reps):
                nc.vector.memset(acc1[:], 0.0)
                nc.vector.memset(acc2[:], 0.0)
                # ================= layer 1 =================
                if rep == 0:
                    MARKS.append(("front1", nc.next_id()))
                layer_front(1, rep)
                if rep == 0:
                    MARKS.append(("sweep1", nc.next_id()))
                edge_sweep(1, rep)
                if rep == 0:
                    MARKS.append(("collective", nc.next_id()))
                if no_collective:
                    with tc.tile_pool(name=f"agcr{rep}", bufs=2) as acp:
                        for cc in range(NCORES):
                            t_ = acp.tile([D, OWN], BF16, tag="agc")
                            nc.sync.dma_start(out=t_[:], in_=ag_in[:])
                            nc.sync.dma_start(out=ag_out[cc * D:(cc + 1) * D, :], in_=t_[:])
                else:
                    nc.gpsimd.collective_compute(
                        "AllGather", mybir.AluOpType.bypass,
                        ins=[ag_in[:]], outs=[ag_out[:]],
                        replica_groups=[list(range(NCORES))])

                # ================= layer 2 =================
                if rep == 0:
                    MARKS.append(("front2", nc.next_id()))
                layer_front(2, rep)
                if rep == 0:
                    MARKS.append(("sweep2", nc.next_id()))
                edge_sweep(2, rep)
    return nc


def make_inputs(edge_index, emb, W1, a_src1, a_dst1, b1, W2, a_src2, a_dst2, b2):
    NW, NPAD, NBUCK, TBL_ROWS = _derived()
    sched, idx_h, R_h, RT_h = prep(edge_index)

    W1 = np.asarray(W1, np.float32)
    a_s1 = np.asarray(a_src1, np.float32)
    a_d1 = np.asarray(a_dst1, np.float32)
    As = np.zeros((D, H1), np.float32)
    Ad = np.zeros((D, H1), np.float32)
    for h in range(H1):
        As[h * C1:(h + 1) * C1, h] = a_s1[h]
        Ad[h * C1:(h + 1) * C1, h] = a_d1[h]
    w1x = np.concatenate([W1, W1 @ As], 1).astype(NPBF16)
    w1d = (W1 @ Ad).astype(NPBF16)
    W2 = np.asarray(W2, np.float32)
    w2x = np.concatenate([W2, W2 @ np.asarray(a_src2, np.float32).T], 1).astype(NPBF16)
    w2d = (W2 @ np.asarray(a_dst2, np.float32).T).astype(NPBF16)

    embT = np.zeros((D, NPAD), NPBF16)
    embT[:, :N] = np.asarray(emb, np.float32).T.astype(NPBF16)
    ident = np.eye(128, dtype=np.float32)
    b1t = np.broadcast_to(np.asarray(b1, np.float32)[None, :], (128, D)).copy()
    b2t = np.broadcast_to(np.asarray(b2, np.float32)[None, :], (128, D)).copy()

    in_maps = []
    for c in range(NCORES):
        in_maps.append({
            "embT": embT, "embTo": np.ascontiguousarray(embT[:, c * OWN:(c + 1) * OWN]),
            "w1aux": w1x, "w1ad": w1d, "w2aux": w2x, "w2ad": w2d,
            "b1t": b1t, "b2t": b2t, "ident": ident,
            "idx16": idx_h[c], "Rh": R_h[c], "RTh": RT_h[c],
        })
    return sched, in_maps


def kernel(edge_index, emb, W1, a_src1, a_dst1, b1, W2, a_src2, a_dst2, b2):
    sched, in_maps = make_inputs(edge_index, emb, W1, a_src1, a_dst1, b1,
                                 W2, a_src2, a_dst2, b2)
    nc = build(sched)
    nc.finalize()
    res = run_bass_kernel_spmd(nc, in_maps, core_ids=list(range(NCORES)))
    out = np.zeros((N, D), np.float32)
    for c in range(NCORES):
        lo, hi = c * OWN, min((c + 1) * OWN, N)
        if lo < N:
            out[lo:hi] = res.results[c]["out_own"][:hi - lo]
    return out


# revision 12
# speedup vs baseline: 1.8274x; 1.4570x over previous
"""Bass/Trainium2 kernel for the 2-layer GAT (nn_GAT_11106785427688).

Strategy (8 NeuronCores, SPMD single NEFF):
- dst-ownership sharding: core c owns nodes [c*OWN, (c+1)*OWN); it receives
  every edge whose dst it owns (~137K edges), so segment-softmax denominators
  and message sums complete locally -- no all-reduce. One AllGather of the
  layer-1 activations between layers; host assembles the final output from
  per-core slices.
- Per-edge gather of packed [h | a_src.h] rows (fp16, 256B) from an HBM table
  via the SWDGE dma_gather custom op (int16 indices -> src buckets of 32768
  rows; table rows permuted so the dense phase writes 2KB-contiguous runs).
- No indexed scatter (HW dma_scatter_add loses duplicate updates): edges are
  grouped by 128-node dst window; the one-hot R [edges x nodes] and its
  transpose RT [nodes x edges] are PRECOMPUTED ON HOST (pure edge-index
  preprocessing) and streamed from HBM, so the DVE never builds one-hots.
  R turns segment-sum into PE matmul accumulated in PSUM; RT gathers the
  per-window a_dst values to edges via PE. Softmax division is deferred:
  out = (sum_e w*h[src]) / (sum_e w).
- exp(leakyrelu(e)) computed without max-subtraction (shift-invariant).
- adw_fill (self-loops + per-window a_dst) is interleaved with the dense
  table build so PE/Act/DVE/DMA overlap instead of running serial phases.
"""
import numpy as np
import ml_dtypes

from concourse import bacc, mybir
import concourse.tile as tile
from concourse.bass_utils import run_bass_kernel_spmd

# ---- problem constants ----
N = 100000
D = 64
H1, C1 = 4, 16
NEG = 0.2
NCORES = 8
OWN = 12544                 # 98 windows * 128 per core
BUCK = 32768
CHUNK = 1024                # gather idxs per dma_gather call (ring limit)
TPC = CHUNK // 128          # tiles per chunk = 8

F16 = mybir.dt.float16
F32 = mybir.dt.float32
BF16 = mybir.dt.bfloat16
I16 = mybir.dt.int16
NPF16 = np.float16
NPBF16 = ml_dtypes.bfloat16

ACT = mybir.ActivationFunctionType


def _derived():
    NW = OWN // 128
    NPAD = NCORES * OWN
    NBUCK = (NPAD + BUCK - 1) // BUCK
    TBL_ROWS = NBUCK * BUCK
    return NW, NPAD, NBUCK, TBL_ROWS


def _perm_row(src):
    """Permuted table row for node src: tb*1024 + p*8 + j (write-friendly)."""
    tb, r = np.divmod(src, 1024)
    j, p = np.divmod(r, 128)
    return tb * 1024 + p * 8 + j


def prep(edge_index):
    """Vectorized host prep: quantile-banded schedule.

    Per-(core,window) edges sorted by src, quantile-spread into the padded
    window group (G_w = roundup128(max-over-cores)). Window tiles are split
    into bands of <=3 tiles; the schedule is band-major so consecutive tiles
    cover the same src-quantile region. Each 1024-slot chunk then spans <=~31
    perm-blocks and gets ONE dma_gather call with a dynamic host-computed
    base (int16 idx). Bands are chunk-aligned (pad tiles trail per band).

    Also builds, per core, the fp16 one-hot streams R (edge-major: used as
    matmul lhsT for the per-window segment sums) and RT (node-major: used as
    lhsT to gather per-window a_dst values to edge positions).
    """
    NW, NPAD, NBUCK, TBL_ROWS = _derived()
    # self-loops are handled densely in adw_fill, not in the gather sweep
    src = np.asarray(edge_index[0])
    dst = np.asarray(edge_index[1])
    owner = dst // OWN

    per_core = []
    counts = np.zeros((NCORES, NW), np.int64)
    for c in range(NCORES):
        m = owner == c
        s = src[m]
        d = dst[m] - c * OWN
        w = d >> 7
        order = np.lexsort((s, w))
        s, d, w = s[order], d[order], w[order]
        per_core.append((s, d, w))
        counts[c] = np.bincount(w, minlength=NW)

    gsize = ((counts.max(0) + 127) // 128 * 128).astype(np.int64)   # [NW]
    kw = gsize // 128                                               # tiles/window

    # band-major tile schedule: band b = quantile quarter [b/4,(b+1)/4) of
    # every window, so run centers align across windows regardless of K_w
    NBANDS = 4
    kb = [[int(round(b * int(kw[w]) / NBANDS)) for b in range(NBANDS + 1)]
          for w in range(NW)]
    tile_list = []          # (w, k) in schedule order
    for b in range(NBANDS):
        for w in range(NW):
            for k in range(kb[w][b], kb[w][b + 1]):
                tile_list.append((w, k))
        # chunk-align each band (pad tiles trail inside the band's last chunk)
        while len(tile_list) % TPC != 0:
            tile_list.append((-1, -1))

    n_tiles = len(tile_list)
    total_slots = n_tiles * 128
    n_chunks = total_slots // CHUNK
    tile_w = np.array([w for w, _ in tile_list], np.int64)
    # slot base of each (w,k) tile
    tile_base = {}
    for t, (w, k) in enumerate(tile_list):
        if w >= 0:
            tile_base[(w, k)] = t * 128
    # first/last per (window, band) run
    tile_first = np.zeros(n_tiles, bool)
    tile_last = np.zeros(n_tiles, bool)
    tile_final = np.zeros(n_tiles, bool)
    for t, (w, k) in enumerate(tile_list):
        if w < 0:
            continue
        tile_first[t] = k in [kb[w][b] for b in range(NBANDS)]
        tile_last[t] = (k + 1) in [kb[w][b + 1] for b in range(NBANDS)]
        tile_final[t] = k + 1 == int(kw[w])

    # per-core slot arrays + per-tile block ranges
    idx_h = np.zeros((NCORES, 128, n_chunks * (CHUNK // 16)), np.int16)
    R_h = np.zeros((NCORES, 128, n_tiles * 128), NPF16)
    RT_h = np.zeros((NCORES, 128, n_tiles * 128), NPF16)
    pr_all = np.zeros((NCORES, total_slots), np.int64)
    off_all = np.full((NCORES, total_slots), -1, np.int64)
    tb_arr = np.full(NW * 32, -1, np.int64)
    for (w, k), sb in tile_base.items():
        tb_arr[w * 32 + k] = sb
    kidx = np.arange(128)
    for c in range(NCORES):
        s, d, w = per_core[c]
        grp_first = np.searchsorted(w, np.arange(NW))
        rank = np.arange(len(s)) - grp_first[w]
        q = (rank * gsize[w]) // counts[c][w]      # quantile-spread in window
        slot = tb_arr[w * 32 + (q // 128)] + (q % 128)
        assert (slot >= 0).all()
        pr_all[c][slot] = _perm_row(s)
        off_all[c][slot] = d & 127
        offs = off_all[c].reshape(n_tiles, 128)
        # R[p, t*128+k] = (off(slot t*128+p) == k); pads (off=-1) -> zero col
        R_h[c] = (offs[:, :, None] == kidx[None, None, :]) \
            .transpose(1, 0, 2).reshape(128, -1).astype(NPF16)
        # RT[p, t*128+e] = (off(slot t*128+e) == p)
        RT_h[c] = (offs[None, :, :] == kidx[:, None, None]) \
            .reshape(128, -1).astype(NPF16)

    # per-chunk gather calls with dynamic base (split if span > 31 blocks)
    real = off_all >= 0
    blk = np.where(real, pr_all // 1024, 1 << 30)
    blk_hi = np.where(real, pr_all // 1024, -1)
    gathers = []
    slot_base = np.zeros(total_slots, np.int64)
    for cidx in range(n_chunks):
        calls = []
        j = 0
        nlive = sum(1 for jj in range(TPC) if tile_w[cidx * TPC + jj] >= 0)
        while j < nlive:
            j0 = j
            s0 = cidx * CHUNK + j0 * 128
            lo = int(blk[:, s0:s0 + 128].min())
            hi = int(blk_hi[:, s0:s0 + 128].max())
            j += 1
            while j < nlive:
                s1 = cidx * CHUNK + j * 128
                nlo = min(lo, int(blk[:, s1:s1 + 128].min()))
                nhi = max(hi, int(blk_hi[:, s1:s1 + 128].max()))
                if nhi - nlo > 31:
                    break
                lo, hi = nlo, nhi
                j += 1
            if lo >= (1 << 30):
                lo = 0
            base = lo * 1024
            calls.append((j0, j - j0, int(base)))
            slot_base[cidx * CHUNK + j0 * 128: cidx * CHUNK + j * 128] = base
        if not calls:
            calls.append((0, TPC, 0))
        gathers.append(calls)

    for c in range(NCORES):
        gi = pr_all[c] - slot_base
        gi[~real[c]] = 0
        assert (gi >= 0).all() and (gi < 32768).all()
        gia = gi.reshape(n_chunks, CHUNK // 16, 16).transpose(0, 2, 1)
        idx_h[c] = np.tile(gia, (1, 8, 1)).transpose(1, 0, 2).reshape(128, -1)

    sched = dict(n_chunks=n_chunks, tile_w=tile_w.tolist(),
                 tile_first=tile_first.tolist(), tile_last=tile_last.tolist(),
                 tile_final=tile_final.tolist(), gathers=gathers)
    return sched, idx_h, R_h, RT_h


MARKS = []


def build(sched, debug=False, no_collective=False, reps=1):
    MARKS.clear()
    NW, NPAD, NBUCK, TBL_ROWS = _derived()
    n_chunks = sched["n_chunks"]
    tile_w = sched["tile_w"]
    tile_first = sched["tile_first"]
    tile_last = sched["tile_last"]
    tile_final = sched["tile_final"]
    gathers = sched["gathers"]
    n_tiles = n_chunks * TPC
    NT_DENSE = NPAD // 128
    NB_DENSE = (NT_DENSE + 7) // 8

    nc = bacc.Bacc(None, target_bir_lowering=False, num_swdge_queues=4)

    embT = nc.dram_tensor("embT", [D, NPAD], BF16, kind="ExternalInput")
    embTo = nc.dram_tensor("embTo", [D, OWN], BF16, kind="ExternalInput")
    w1aux = nc.dram_tensor("w1aux", [D, D + H1], BF16, kind="ExternalInput")
    w1ad = nc.dram_tensor("w1ad", [D, H1], BF16, kind="ExternalInput")
    w2aux = nc.dram_tensor("w2aux", [D, D + 1], BF16, kind="ExternalInput")
    w2ad = nc.dram_tensor("w2ad", [D, 1], BF16, kind="ExternalInput")
    b1t_in = nc.dram_tensor("b1t", [128, D], F32, kind="ExternalInput")
    b2t_in = nc.dram_tensor("b2t", [128, D], F32, kind="ExternalInput")
    ident_in = nc.dram_tensor("ident", [128, 128], F32, kind="ExternalInput")
    idx_in = nc.dram_tensor("idx16", [128, n_chunks * (CHUNK // 16)], I16, kind="ExternalInput")
    R_in = nc.dram_tensor("Rh", [128, n_tiles * 128], F16, kind="ExternalInput")
    RT_in = nc.dram_tensor("RTh", [128, n_tiles * 128], F16, kind="ExternalInput")
    out_own = nc.dram_tensor("out_own", [OWN, D], F32, kind="ExternalOutput")

    table = nc.dram_tensor("table", [TBL_ROWS, 128], F16)
    ag_in = nc.dram_tensor("ag_in", [D, OWN], BF16)
    ag_out = nc.dram_tensor("ag_out", [NCORES * D, OWN], BF16, addr_space="Shared")

    with tile.TileContext(nc) as tc:
        with tc.tile_pool(name="persist", bufs=1) as pp:
            b1t = pp.tile([128, D], F32)
            b2t = pp.tile([128, D], F32)
            ident = pp.tile([128, 128], F32)
            w1x = pp.tile([D, D + H1], BF16)
            w1d = pp.tile([D, H1], BF16)
            w2x = pp.tile([D, D + 1], BF16)
            w2d = pp.tile([D, 1], BF16)
            idx_s = pp.tile([128, n_chunks * (CHUNK // 16)], I16)
            adw = pp.tile([128, NW * H1], F16)
            adw2 = pp.tile([128, NW], F16)
            acc1 = pp.tile([128, NW * (D + H1)], F32)
            acc2 = pp.tile([128, NW * (D + 1)], F32)
            for t_, s_ in [(b1t, b1t_in), (b2t, b2t_in), (ident, ident_in),
                           (w1x, w1aux), (w1d, w1ad), (w2x, w2aux), (w2d, w2ad),
                           (idx_s, idx_in)]:
                nc.sync.dma_start(out=t_[:], in_=s_[:])

            def layer_front(layer, rep):
                """Interleaved adw_fill + dense table build.

                adw: per owned window, a_dst.h via x_own @ (W @ Ad) plus the
                dense self-loop contribution (e_self = asrc.h + adst.h,
                acc += [w*h | w]).  dense: x @ Waux -> fp16 table rows
                (permuted layout).  Emitted interleaved under shared pools so
                Tile overlaps them across engines.
                """
                wad = w1d if layer == 1 else w2d
                waux = w1x if layer == 1 else w2x
                H = H1 if layer == 1 else 1
                CH = C1 if layer == 1 else D
                EC = D + H
                ncol = EC
                acc = acc1 if layer == 1 else acc2
                dst_t = adw if layer == 1 else adw2
                srcT = embTo if layer == 1 else ag_in
                with tc.tile_pool(name=f"aw{layer}r{rep}", bufs=3) as ap, \
                     tc.tile_pool(name=f"awp{layer}r{rep}", bufs=3, space="PSUM") as app:

                    def adw_step(w, ltb):
                        lt = ltb[:, (w % 8) * 128:(w % 8 + 1) * 128]
                        ps = app.tile([128, H], F32, tag="p")
                        nc.tensor.matmul(out=ps[:], lhsT=lt, rhs=wad[:],
                                         start=True, stop=True)
                        nc.scalar.activation(out=dst_t[:, w * H:(w + 1) * H], in_=ps[:],
                                             func=ACT.Copy)
                        psh = app.tile([128, EC], F32, tag="h")
                        nc.tensor.matmul(out=psh[:], lhsT=lt, rhs=waux[:],
                                         start=True, stop=True)
                        ho = ap.tile([128, EC], F16, tag="h16")
                        nc.scalar.activation(out=ho[:], in_=psh[:], func=ACT.Copy)
                        es = ap.tile([128, H], F32, tag="es")
                        nc.vector.tensor_tensor(out=es[:], in0=ho[:, D:D + H],
                                                in1=dst_t[:, w * H:(w + 1) * H],
                                                op=mybir.AluOpType.add)
                        lrs = ap.tile([128, H], F32, tag="lrs")
                        nc.vector.tensor_scalar_mul(out=lrs[:], in0=es[:], scalar1=NEG)
                        nc.vector.tensor_tensor(out=lrs[:], in0=lrs[:], in1=es[:],
                                                op=mybir.AluOpType.max)
                        wx = ap.tile([128, D], F16, tag="wx")
                        wx3 = wx[:].rearrange("p (h k) -> p h k", k=CH)
                        nc.scalar.activation(
                            out=wx3,
                            in_=lrs[:, :, None].to_broadcast([128, H, CH]),
                            func=ACT.Exp)
                        ms = ap.tile([128, D], F32, tag="ms")
                        nc.vector.tensor_tensor(out=ms[:], in0=ho[:, 0:D], in1=wx[:],
                                                op=mybir.AluOpType.mult)
                        nc.vector.tensor_tensor(
                            out=acc[:, w * EC:w * EC + D],
                            in0=acc[:, w * EC:w * EC + D], in1=ms[:],
                            op=mybir.AluOpType.add)
                        nc.vector.tensor_tensor(
                            out=acc[:, w * EC + D:(w + 1) * EC],
                            in0=acc[:, w * EC + D:(w + 1) * EC],
                            in1=wx3[:, :, 0],
                            op=mybir.AluOpType.add)

                    ltb = None
                    for w in range(NW):
                        if w % 8 == 0:
                            nwb = min(8, NW - w)
                            ltb = ap.tile([D, 8 * 128], BF16, tag="lb")
                            nc.sync.dma_start(
                                out=ltb[:, 0:nwb * 128],
                                in_=srcT[:, w * 128:(w + nwb) * 128])
                        adw_step(w, ltb)

                with tc.tile_pool(name=f"dns{layer}r{rep}", bufs=3) as dp, \
                     tc.tile_pool(name=f"dnp{layer}r{rep}", bufs=3, space="PSUM") as dpp:

                    def dense_step(tb0):
                        nb = min(2, NB_DENSE - tb0)
                        lt = dp.tile([D, 2 * 1024], BF16, tag="lhs")
                        if layer == 1:
                            nc.sync.dma_start(
                                out=lt[:, 0:nb * 1024],
                                in_=embT[:, tb0 * 1024:(tb0 + nb) * 1024])
                        else:
                            # global tiles -> (core, window) runs
                            j = 0
                            while j < 8 * nb:
                                t = tb0 * 8 + j
                                co, wl = divmod(t, NW)
                                nrun = min(8 * nb - j, NW - wl)
                                nc.sync.dma_start(
                                    out=lt[:, j * 128:(j + nrun) * 128],
                                    in_=ag_out[co * D:(co + 1) * D,
                                               wl * 128:(wl + nrun) * 128])
                                j += nrun
                        stg = dp.tile([128, 2 * 1024], F16, tag="stg")
                        for j in range(8 * nb):
                            ps = dpp.tile([128, ncol], F32, tag="d")
                            nc.tensor.matmul(out=ps[:], lhsT=lt[:, j * 128:(j + 1) * 128],
                                             rhs=waux[:], start=True, stop=True)
                            if j % 2 == 0:
                                nc.scalar.activation(
                                    out=stg[:, j * 128:j * 128 + ncol], in_=ps[:],
                                    func=ACT.Copy)
                            else:
                                nc.vector.tensor_copy(
                                    out=stg[:, j * 128:j * 128 + ncol], in_=ps[:])
                        for b in range(nb):
                            nc.sync.dma_start(
                                out=table[(tb0 + b) * 1024:(tb0 + b + 1) * 1024]
                                .rearrange("(p j) k -> p (j k)", j=8),
                                in_=stg[:, b * 1024:(b + 1) * 1024])

                    for tb0 in range(0, NB_DENSE, 2):
                        dense_step(tb0)

            def fin1(w, fp, fpp):
                """Finalize window w of layer 1: softmax div, bias, ELU,
                transpose, write ag_in column block."""
                EC = D + H1
                den = fp.tile([128, H1], F32, tag="den")
                nc.vector.tensor_scalar_add(
                    out=den[:], in0=acc1[:, w * EC + D:(w + 1) * EC], scalar1=1e-16)
                rec = fp.tile([128, H1], F32, tag="rec")
                nc.vector.reciprocal(out=rec[:], in_=den[:])
                x2 = fp.tile([128, D], F32, tag="x2")
                nc.vector.tensor_tensor(
                    out=x2[:].rearrange("p (h k) -> p h k", k=C1),
                    in0=acc1[:, w * EC:w * EC + D].rearrange("p (h k) -> p h k", k=C1),
                    in1=rec[:, :, None].to_broadcast([128, H1, C1]),
                    op=mybir.AluOpType.mult)
                nc.vector.tensor_tensor(out=x2[:], in0=x2[:], in1=b1t[:],
                                        op=mybir.AluOpType.add)
                # elu(x) = relu(x) - relu(1 - exp(x))
                ex = fp.tile([128, D], F32, tag="ex")
                nc.scalar.activation(out=ex[:], in_=x2[:], func=ACT.Exp)
                u = fp.tile([128, D], F32, tag="u")
                nc.scalar.activation(out=u[:], in_=ex[:], func=ACT.Relu,
                                     scale=-1.0, bias=1.0)
                r = fp.tile([128, D], F32, tag="r")
                nc.scalar.activation(out=r[:], in_=x2[:], func=ACT.Relu)
                xe = fp.tile([128, D], F32, tag="xe")
                nc.vector.tensor_tensor(out=xe[:], in0=r[:], in1=u[:],
                                        op=mybir.AluOpType.subtract)
                pst = fpp.tile([D, 128], F32, tag="t")
                nc.tensor.transpose(out=pst[:], in_=xe[:], identity=ident[:])
                xt = fp.tile([D, 128], BF16, tag="xt")
                nc.scalar.activation(out=xt[:], in_=pst[:], func=ACT.Copy)
                nc.sync.dma_start(out=ag_in[:, w * 128:(w + 1) * 128], in_=xt[:])

            def fin2(w, fp):
                """Finalize window w of layer 2: softmax div, bias, l2-norm,
                write out_own rows."""
                EC = D + 1
                den = fp.tile([128, 1], F32, tag="den")
                nc.vector.tensor_scalar_add(
                    out=den[:], in0=acc2[:, w * EC + D:(w + 1) * EC], scalar1=1e-16)
                rec = fp.tile([128, 1], F32, tag="rec")
                nc.vector.reciprocal(out=rec[:], in_=den[:])
                o = fp.tile([128, D], F32, tag="o")
                nc.vector.tensor_tensor(
                    out=o[:], in0=acc2[:, w * EC:w * EC + D],
                    in1=rec[:].to_broadcast([128, D]), op=mybir.AluOpType.mult)
                nc.vector.tensor_tensor(out=o[:], in0=o[:], in1=b2t[:],
                                        op=mybir.AluOpType.add)
                sq = fp.tile([128, D], F32, tag="sq")
                ss = fp.tile([128, 1], F32, tag="ss")
                nc.scalar.activation(out=sq[:], in_=o[:], func=ACT.Square,
                                     accum_out=ss[:])
                nrm = fp.tile([128, 1], F32, tag="nr")
                nc.scalar.activation(out=nrm[:], in_=ss[:], func=ACT.Sqrt)
                nc.vector.tensor_scalar_max(out=nrm[:], in0=nrm[:], scalar1=1e-12)
                rn = fp.tile([128, 1], F32, tag="rn")
                nc.vector.reciprocal(out=rn[:], in_=nrm[:])
                of = fp.tile([128, D], F32, tag="of")
                nc.vector.tensor_tensor(out=of[:], in0=o[:],
                                        in1=rn[:].to_broadcast([128, D]),
                                        op=mybir.AluOpType.mult)
                nc.sync.dma_start(out=out_own[w * 128:(w + 1) * 128, :], in_=of[:])

            def edge_sweep(layer, rep):
                H = H1 if layer == 1 else 1
                CH = C1 if layer == 1 else D
                EC = D + H
                acc = acc1 if layer == 1 else acc2
                adwl = adw if layer == 1 else adw2
                with tc.tile_pool(name=f"eg{layer}r{rep}", bufs=10) as gp, \
                     tc.tile_pool(name=f"er{layer}r{rep}", bufs=8) as rp, \
                     tc.tile_pool(name=f"em{layer}r{rep}", bufs=8) as mp, \
                     tc.tile_pool(name=f"fw{layer}r{rep}", bufs=3) as fwp, \
                     tc.tile_pool(name=f"epa{layer}r{rep}", bufs=2, space="PSUM") as pap, \
                     tc.tile_pool(name=f"fwp{layer}r{rep}", bufs=2, space="PSUM") as fpp, \
                     tc.tile_pool(name=f"epg{layer}r{rep}", bufs=4, space="PSUM") as pgp:
                    group_ps = {}
                    gq = 0
                    for c in range(n_chunks):
                        live = [j for j in range(TPC) if tile_w[c * TPC + j] >= 0]
                        assert live == list(range(len(live))), "pads must trail"
                        nl = len(live)
                        # one-hot streams from host (R: edge-major, RT: node-major);
                        # emitted before the gather so they prefetch ahead
                        if live:
                            Rt = rp.tile([128, TPC * 128], F16, tag="R")
                            nc.sync.dma_start(
                                out=Rt[:, 0:nl * 128],
                                in_=R_in[:, (c * TPC) * 128:(c * TPC + nl) * 128])
                            RTt = rp.tile([128, TPC * 128], F16, tag="RT")
                            nc.sync.dma_start(
                                out=RTt[:, 0:nl * 128],
                                in_=RT_in[:, (c * TPC) * 128:(c * TPC + nl) * 128])
                        ght = gp.tile([128, TPC * 128], F16, tag="ght")
                        ght3g = ght[:].rearrange("p (a k) -> p a k", k=128)
                        for (j0, ntl, base) in gathers[c]:
                            hi = min(base + 32768, TBL_ROWS)
                            cb = c * (CHUNK // 16)
                            nc.gpsimd.dma_gather(
                                ght3g[:, j0:j0 + ntl, :],
                                table[base:hi, :],
                                idx_s[:, cb + j0 * 8:cb + (j0 + ntl) * 8],
                                ntl * 128, ntl * 128, 128, elem_step=128,
                                queue_num=gq % 4)
                            gq += 1
                        if not live:
                            continue
                        ght3 = ght[:].rearrange("p (a k) -> p a k", k=128)
                        # per-edge a_dst via PE gather: psa = RT_j^T @ adw_w
                        psa = pap.tile([128, nl * H], F32, tag="a", name=f"psa{c}")
                        for j in live:
                            w = tile_w[c * TPC + j]
                            nc.tensor.matmul(
                                out=psa[:, j * H:(j + 1) * H],
                                lhsT=RTt[:, j * 128:(j + 1) * 128],
                                rhs=adwl[:, w * H:(w + 1) * H],
                                start=True, stop=True)
                        ew = mp.tile([128, nl * H], F32, tag="ew", name=f"ew{c}")
                        nc.vector.tensor_tensor(
                            out=ew[:].rearrange("p (a h) -> p a h", h=H),
                            in0=psa[:].rearrange("p (a h) -> p a h", h=H),
                            in1=ght3[:, 0:nl, D:D + H],
                            op=mybir.AluOpType.add)
                        lr = mp.tile([128, nl * H], F32, tag="lr", name=f"lr{c}")
                        nc.vector.tensor_scalar_mul(out=lr[:], in0=ew[:], scalar1=NEG)
                        nc.vector.tensor_tensor(out=lr[:], in0=lr[:], in1=ew[:],
                                                op=mybir.AluOpType.max)
                        # exp, pre-expanded across the C dim (Act) -> DVE mult
                        we16 = mp.tile([128, nl * D], F16, tag="we", name=f"we{c}")
                        we4 = we16[:].rearrange("p (a h k) -> p a h k", h=H, k=CH)
                        nc.scalar.activation(
                            out=we4,
                            in_=lr[:].rearrange("p (a h) -> p a h", h=H)[:, :, :, None]
                                .to_broadcast([128, nl, H, CH]),
                            func=ACT.Exp)
                        msgt = mp.tile([128, nl * EC], F16, tag="msg", name=f"msg{c}")
                        msgt3 = msgt[:].rearrange("p (a k) -> p a k", k=EC)
                        # denominator cols written directly by a second exp (Act)
                        nc.scalar.activation(
                            out=msgt3[:, :, D:D + H],
                            in_=lr[:].rearrange("p (a h) -> p a h", h=H),
                            func=ACT.Exp)
                        nc.vector.tensor_tensor(
                            out=msgt3[:, :, 0:D].rearrange("p a (h k) -> p a h k", k=CH),
                            in0=ght3[:, 0:nl, 0:D].rearrange("p a (h k) -> p a h k", k=CH),
                            in1=we4,
                            op=mybir.AluOpType.mult)
                        for j in live:
                            t = c * TPC + j
                            w = tile_w[t]
                            if tile_first[t]:
                                group_ps[w] = pgp.tile([128, EC], F32, tag="g", name=f"grp{w}")
                            ps = group_ps[w]
                            nc.tensor.matmul(
                                out=ps[:], lhsT=Rt[:, j * 128:(j + 1) * 128],
                                rhs=msgt[:, j * EC:(j + 1) * EC],
                                start=tile_first[t], stop=tile_last[t])
                            if tile_last[t]:
                                nc.vector.tensor_tensor(
                                    out=acc[:, w * EC:(w + 1) * EC],
                                    in0=acc[:, w * EC:(w + 1) * EC],
                                    in1=ps[:], op=mybir.AluOpType.add)
                                del group_ps[w]
                                if tile_final[t]:
                                    if layer == 1:
                                        fin1(w, fwp, fpp)
                                    else:
                                        fin2(w, fwp)
                    assert not group_ps

            for rep in range(reps):
                nc.vector.memset(acc1[:], 0.0)
                nc.vector.memset(acc2[:], 0.0)
                # ================= layer 1 =================
                if rep == 0:
                    MARKS.append(("front1", nc.next_id()))
                layer_front(1, rep)
                if rep == 0:
                    MARKS.append(("sweep1", nc.next_id()))
                edge_sweep(1, rep)
                if rep == 0:
                    MARKS.append(("collective", nc.next_id()))
                if no_collective:
                    with tc.tile_pool(name=f"agcr{rep}", bufs=2) as acp:
                        for cc in range(NCORES):
                            t_ = acp.tile([D, OWN], BF16, tag="agc")
                            nc.sync.dma_start(out=t_[:], in_=ag_in[:])
                            nc.sync.dma_start(out=ag_out[cc * D:(cc + 1) * D, :], in_=t_[:])
                else:
                    nc.gpsimd.collective_compute(
                        "AllGather", mybir.AluOpType.bypass,
                        ins=[ag_in[:]], outs=[ag_out[:]],
                        replica_groups=[list(range(NCORES))])

                # ================= layer 2 =================
                if rep == 0:
                    MARKS.append(("front2", nc.next_id()))
                layer_front(2, rep)
                if rep == 0:
                    MARKS.append(("sweep2", nc.next_id()))
                edge_sweep(2, rep)
    return nc


def make_inputs(edge_index, emb, W1, a_src1, a_dst1, b1, W2, a_src2, a_dst2, b2):
    NW, NPAD, NBUCK, TBL_ROWS = _derived()
    sched, idx_h, R_h, RT_h = prep(edge_index)

    W1 = np.asarray(W1, np.float32)
    a_s1 = np.asarray(a_src1, np.float32)
    a_d1 = np.asarray(a_dst1, np.float32)
    As = np.zeros((D, H1), np.float32)
    Ad = np.zeros((D, H1), np.float32)
    for h in range(H1):
        As[h * C1:(h + 1) * C1, h] = a_s1[h]
        Ad[h * C1:(h + 1) * C1, h] = a_d1[h]
    w1x = np.concatenate([W1, W1 @ As], 1).astype(NPBF16)
    w1d = (W1 @ Ad).astype(NPBF16)
    W2 = np.asarray(W2, np.float32)
    w2x = np.concatenate([W2, W2 @ np.asarray(a_src2, np.float32).T], 1).astype(NPBF16)
    w2d = (W2 @ np.asarray(a_dst2, np.float32).T).astype(NPBF16)

    embT = np.zeros((D, NPAD), NPBF16)
    embT[:, :N] = np.asarray(emb, np.float32).T.astype(NPBF16)
    ident = np.eye(128, dtype=np.float32)
    b1t = np.broadcast_to(np.asarray(b1, np.float32)[None, :], (128, D)).copy()
    b2t = np.broadcast_to(np.asarray(b2, np.float32)[None, :], (128, D)).copy()

    in_maps = []
    for c in range(NCORES):
        in_maps.append({
            "embT": embT, "embTo": np.ascontiguousarray(embT[:, c * OWN:(c + 1) * OWN]),
            "w1aux": w1x, "w1ad": w1d, "w2aux": w2x, "w2ad": w2d,
            "b1t": b1t, "b2t": b2t, "ident": ident,
            "idx16": idx_h[c], "Rh": R_h[c], "RTh": RT_h[c],
        })
    return sched, in_maps


def kernel(edge_index, emb, W1, a_src1, a_dst1, b1, W2, a_src2, a_dst2, b2):
    sched, in_maps = make_inputs(edge_index, emb, W1, a_src1, a_dst1, b1,
                                 W2, a_src2, a_dst2, b2)
    nc = build(sched)
    nc.finalize()
    res = run_bass_kernel_spmd(nc, in_maps, core_ids=list(range(NCORES)))
    out = np.zeros((N, D), np.float32)
    for c in range(NCORES):
        lo, hi = c * OWN, min((c + 1) * OWN, N)
        if lo < N:
            out[lo:hi] = res.results[c]["out_own"][:hi - lo]
    return out
